# revision 24
# baseline (speedup 1.0000x reference)
"""Trainium2 Bass kernel for nn_Attention (B=4, N=1024, C=1024, H=16, hd=64).

Sharding: 8 cores; core c handles batch b=c//2 and heads (c%2)*8..(c%2)*8+8.
Each core computes qkv for its head slice from x[b]^T, full attention for its
8 heads, and a partial output projection over its 512 head-channels.  The
device writes E = exp(S*scale) (bf16) plus per-row sums D; the host finishes
attn = E * mask / D (it has the mask and D).  The device's own O = P @ V gets
the mask fused into the P^T copyback (mask^T) and 1/D folded into the O^T
copyback (broadcast recip tile built via a tiny PE transpose).

Phase 2 is an explicit software pipeline at head granularity: step s emits
head s's S-matmuls + exp while emitting head s-2's P-transposes and O^T
accumulation, keeping the PE stream dense.

Outputs (out, attn) exactly as the reference:
    out  [4, 1024, 1024] f32
    attn [4, 16, 1024, 1024] f32  (post-softmax, hard-masked)
"""

import numpy as np
import ml_dtypes

import concourse.bass as bass
import concourse.mybir as mybir
import concourse.tile as tile
from concourse import bacc
from concourse.bass_utils import run_bass_kernel_spmd
from concourse.masks import make_identity

B, N, C = 4, 1024, 1024
HEADS = 16
HD = 64
SCALE = HD ** -0.5
NCORES = 8
HPC = 8  # heads per core
P = 128

F32 = mybir.dt.float32
F32R = mybir.dt.float32r
BF16 = mybir.dt.bfloat16
MULT = mybir.AluOpType.mult
ADD = mybir.AluOpType.add
EXP = mybir.ActivationFunctionType.Exp

_compiled = None


def _body(tc, xT, w_qk, w_v, b_qk, b_v, maskTd, w_proj, attn_out, d_out, y_out, ctx):
    nc = tc.nc

    singles = ctx.enter_context(tc.tile_pool(name="singles", bufs=1))
    small = ctx.enter_context(tc.tile_pool(name="small", bufs=32))

    qkt_sb = singles.tile([P, 8, N], F32R)    # rows: chunks 0-3 = Q^T, 4-7 = K^T
    v_sb = singles.tile([P, 8, 512], BF16)    # V natural: [k within chunk, kc, v']
    maskT_sb = singles.tile([P, 8, N], BF16)  # mask^T: [k within chunk, kc, q]
    wproj_sb = singles.tile([P, 4, C], BF16)
    ot_sb = singles.tile([P, 4, N], BF16)     # O^T per head-pair: [c in pair, hp, q]
    bqk_sb = singles.tile([P, 8], F32)
    bv_sb = singles.tile([P, 512], F32)
    d_sb = singles.tile([P, 8, 8], F32)       # rowsums: [q%128, head, qchunk]
    r_sb = singles.tile([P, 8, 8], F32)       # 1/rowsum, same layout
    ident = singles.tile([P, P], BF16)
    identf = singles.tile([P, P], F32)
    make_identity(nc, ident[:])
    make_identity(nc, identf[:])

    ph1 = ctx.enter_context(tc.tile_pool(name="ph1", bufs=1))
    xT_sb = ph1.tile([P, 8, N], F32R)
    wv_sb = ph1.tile([P, 8, 512], F32R)

    # ---------- phase 1: [Q^T; K^T] = (x @ w_qk)^T : lhsT = w_qk, rhs = x^T
    # cc-outer over 8 psum banks so compute starts right after the first chunk
    with tc.tile_pool(name="ph1w", bufs=1) as ph1w, \
         tc.tile_pool(name="ph1ps", bufs=8, space="PSUM") as ph1ps:
        wqk_sb = ph1w.tile([P, 8, 1024], F32R)
        # chunked loads so the first matmuls start after ~1MB, not 14MB
        xT_r = xT.rearrange("(cc p) n -> p cc n", p=P)
        wqk_r = w_qk.rearrange("(cc p) o -> p cc o", p=P)
        wv_r = w_v.rearrange("(cc p) o -> p cc o", p=P)
        for cc in range(8):
            nc.sync.dma_start(wqk_sb[:, cc, :], wqk_r[:, cc, :])
            nc.sync.dma_start(xT_sb[:, cc, :], xT_r[:, cc, :])
            nc.sync.dma_start(wv_sb[:, cc, :], wv_r[:, cc, :])
        nc.sync.dma_start(bqk_sb[:], b_qk.rearrange("(cc p) -> p cc", p=P))
        bv_bc_src = bass.AP(tensor=b_v.tensor, offset=b_v.offset,
                            ap=[[0, P]] + [list(a) for a in b_v.ap])
        nc.sync.dma_start(bv_sb[:], bv_bc_src)
        nc.sync.dma_start(maskT_sb[:], maskTd.rearrange("(kc p) q -> p kc q", p=P))
        nc.sync.dma_start(wproj_sb[:], w_proj.rearrange("(hp p) c -> p hp c", p=P))

        for cog in range(2):
            cos = (0, 4, 1, 5) if cog == 0 else (2, 6, 3, 7)
            pss = {}
            for ci in range(4):
                for nh in range(2):
                    pss[(ci, nh)] = ph1ps.tile([P, 512], F32, tag="qk",
                                               name=f"qkps_{cog}_{ci}_{nh}")
            for cc in range(8):
                for ci in range(4):
                    co = cos[ci]
                    for nh in range(2):
                        nc.tensor.matmul(pss[(ci, nh)][:],
                                         wqk_sb[:, cc, co * 128:(co + 1) * 128],
                                         xT_sb[:, cc, nh * 512:(nh + 1) * 512],
                                         start=(cc == 0), stop=(cc == 7))
            for ci in range(4):
                co = cos[ci]
                for nh in range(2):
                    nc.scalar.activation(qkt_sb[:, co, nh * 512:(nh + 1) * 512],
                                         pss[(ci, nh)][:],
                                         mybir.ActivationFunctionType.Identity,
                                         bias=bqk_sb[:, co:co + 1])

    # ---------- phase 2: software-pipelined attention ----------
    p_pool = ctx.enter_context(tc.tile_pool(name="p", bufs=3))
    pt_pool = ctx.enter_context(tc.tile_pool(name="pt", bufs=2))
    rp_pool = ctx.enter_context(tc.tile_pool(name="rp", bufs=2))
    rt_pool = ctx.enter_context(tc.tile_pool(name="rt", bufs=2))
    rtd_pool = ctx.enter_context(tc.tile_pool(name="rtd", bufs=2, space="DRAM"))
    y_pool = ctx.enter_context(tc.tile_pool(name="y", bufs=2))
    s_psum = ctx.enter_context(tc.tile_pool(name="sps", bufs=2, space="PSUM"))
    tr_psum = ctx.enter_context(tc.tile_pool(name="trps", bufs=2, space="PSUM"))
    o_psum = ctx.enter_context(tc.tile_pool(name="ops", bufs=2, space="PSUM"))

    p_tiles = {}
    ops_ = {}
    rp_tiles = {}

    def emit_s_stage(hh, qc):
        """S = Q_h K_h^T for one query chunk; E = exp(S*scale) -> P tile + HBM."""
        sps = s_psum.tile([P, N], F32, tag="sps", name=f"s_{hh}_{qc}")
        rbase = (hh % 2) * 64
        lhsT = qkt_sb[rbase:rbase + 64, hh // 2, qc * 128:(qc + 1) * 128]
        for kh in range(2):
            nc.tensor.matmul(sps[:, kh * 512:(kh + 1) * 512], lhsT,
                             qkt_sb[rbase:rbase + 64, 4 + hh // 2,
                                    kh * 512:(kh + 1) * 512],
                             start=True, stop=True)
        nc.scalar.activation(p_tiles[hh][:, qc, :], sps[:], EXP, scale=SCALE,
                             accum_out=d_sb[:, hh, qc:qc + 1])
        nc.sync.dma_start(attn_out[hh, qc * 128:(qc + 1) * 128, :],
                          p_tiles[hh][:, qc, :])

    def emit_r_stage(hh):
        """1/rowsum for head hh; ship D to HBM (h, p, qc layout: contiguous)."""
        nc.vector.reciprocal(r_sb[:, hh, :], d_sb[:, hh, :])
        nc.sync.dma_start(d_out[hh], d_sb[:, hh, :])

    def emit_rp_build(hp):
        """Broadcast recip tile [c-in-pair, q] for pair hp via PE transpose."""
        rt_ps = s_psum.tile([16, P], F32, tag="sps", name=f"rtps_{hp}")
        nc.tensor.transpose(rt_ps[:], r_sb[:, 2 * hp:2 * hp + 2, :], identf[:])
        rt = rt_pool.tile([16, P], F32, tag="rt", name=f"rt_{hp}")
        nc.scalar.copy(rt[:], rt_ps[:])
        rtd = rtd_pool.tile([16, P], F32, tag="rtd", name=f"rtd_{hp}")
        nc.sync.dma_start(rtd[:], rt[:])
        rp = rp_pool.tile([P, N], F32, tag="rp", name=f"rp_{hp}")
        for hl in (0, 1):
            for qc in range(8):
                src = rtd[hl * 8 + qc:hl * 8 + qc + 1, :]
                bsrc = bass.AP(tensor=src.tensor, offset=src.offset,
                               ap=[[0, 64]] + [list(a) for a in src.ap[1:]])
                nc.sync.dma_start(rp[hl * 64:hl * 64 + 64, qc * 128:(qc + 1) * 128],
                                  bsrc)
        rp_tiles[hp] = rp

    def emit_t_stage(hh, kc):
        """Transpose E_hh's kc block; fuse mask^T into the copyback; O^T accum."""
        hp = hh // 2
        if hh % 2 == 0 and kc == 0:
            ops_[hp] = {half: o_psum.tile([P, 512], F32, tag="otps",
                                          name=f"otps_{hp}_{half}")
                        for half in (0, 1)}
        pt = pt_pool.tile([P, N], BF16, tag="pt", name=f"pt_{hh}_{kc}")
        for qq in range(2):
            trp = tr_psum.tile([P, 512], F32, tag="trp", name=f"trp_{hh}_{kc}_{qq}")
            for j in range(4):
                qc = qq * 4 + j
                # transpose as a REGULAR matmul (E_chunk^T = lhsT.T @ I):
                # transpose-mode ops don't count as PE activity for the HAM
                # clock governor and keep the PE throttled at 1.2 GHz.
                nc.tensor.matmul(trp[:, j * 128:(j + 1) * 128],
                                 p_tiles[hh][:, qc, kc * 128:(kc + 1) * 128],
                                 ident[:], start=True, stop=True)
            # P^T = E^T * mask^T, fused into the PSUM->SBUF copyback
            nc.vector.tensor_tensor(pt[:, qq * 512:(qq + 1) * 512], trp[:],
                                    maskT_sb[:, kc, qq * 512:(qq + 1) * 512], MULT)
        obase = (hh % 2) * 64
        for half in (0, 1):
            for dh in (0, 1):
                cb = obase + dh * 32
                nc.tensor.matmul(ops_[hp][half][cb:cb + 32, :],
                                 v_sb[:, kc, hh * 64 + dh * 32:hh * 64 + dh * 32 + 32],
                                 pt[:, half * 512:(half + 1) * 512],
                                 start=(kc == 0), stop=(kc == 7),
                                 tile_position=(0, cb))
        if hh % 2 == 1 and kc == 7:
            rp = rp_tiles.pop(hp)
            for half in (0, 1):
                # fold 1/rowsum into the copyback: ot = psum * rp
                nc.vector.tensor_tensor(ot_sb[:, hp, half * 512:(half + 1) * 512],
                                        ops_[hp][half][:],
                                        rp[:, half * 512:(half + 1) * 512], MULT)
            del ops_[hp]

    for s in range(HPC + 2):
        if s < HPC:
            p_tiles[s] = p_pool.tile([P, 8, N], BF16, tag="P", name=f"P_{s}")
        if s >= 2 and (s - 2) % 2 == 0:
            emit_rp_build((s - 2) // 2)
        for qc in range(8):
            if s < HPC:
                emit_s_stage(s, qc)
            if s == 0:
                # V natural = x @ w_v (fills head 0's S-phase PE stalls)
                vps = o_psum.tile([P, 512], F32, tag="otps", name=f"vps_{qc}")
                for cc in range(8):
                    nc.tensor.matmul(vps[:], xT_sb[:, cc, qc * 128:(qc + 1) * 128],
                                     wv_sb[:, cc, :],
                                     start=(cc == 0), stop=(cc == 7))
                nc.vector.tensor_tensor(v_sb[:, qc, :], vps[:], bv_sb[:], ADD)
            if s >= 2:
                emit_t_stage(s - 2, qc)
        if s < HPC:
            emit_r_stage(s)

    # ---------- phase 3: output projection (partial over this core's 512 ch) ----------
    for ncc in range(8):
        for ch in range(2):
            yps = o_psum.tile([P, 512], F32, tag="otps", name=f"yps_{ncc}_{ch}")
            for hp in range(4):
                nc.tensor.matmul(yps[:], ot_sb[:, hp, ncc * 128:(ncc + 1) * 128],
                                 wproj_sb[:, hp, ch * 512:(ch + 1) * 512],
                                 start=(hp == 0), stop=(hp == 3))
            yt = y_pool.tile([P, 512], F32, tag="yt", name=f"yt_{ncc}_{ch}")
            nc.scalar.copy(yt[:], yps[:])
            nc.sync.dma_start(y_out[ncc * 128:(ncc + 1) * 128, ch * 512:(ch + 1) * 512],
                              yt[:])


def _build():
    from contextlib import ExitStack
    nc = bacc.Bacc("TRN2", target_bir_lowering=False, debug=False)
    xT = nc.dram_tensor("xT", [C, N], F32R, kind="ExternalInput").ap()
    w_qk = nc.dram_tensor("w_qk", [C, 1024], F32R, kind="ExternalInput").ap()
    w_v = nc.dram_tensor("w_v", [C, 512], F32R, kind="ExternalInput").ap()
    b_qk = nc.dram_tensor("b_qk", [1024], F32, kind="ExternalInput").ap()
    b_v = nc.dram_tensor("b_v", [512], F32, kind="ExternalInput").ap()
    maskT = nc.dram_tensor("maskT", [N, N], BF16, kind="ExternalInput").ap()
    w_proj = nc.dram_tensor("w_proj", [512, C], BF16, kind="ExternalInput").ap()
    attn_out = nc.dram_tensor("attn_out", [HPC, N, N], BF16, kind="ExternalOutput").ap()
    d_out = nc.dram_tensor("d_out", [HPC, P, 8], F32, kind="ExternalOutput").ap()
    y_out = nc.dram_tensor("y_out", [N, C], F32, kind="ExternalOutput").ap()

    with tile.TileContext(nc) as tc, ExitStack() as ctx:
        _body(tc, xT, w_qk, w_v, b_qk, b_v, maskT, w_proj, attn_out, d_out, y_out, ctx)
    nc.compile()
    return nc


def _get_nc():
    global _compiled
    if _compiled is None:
        _compiled = _build()
    return _compiled


def _in_maps(x, attn_mask, w_qkv, b_qkv, w_proj, b_proj):
    x = np.asarray(x, np.float32)
    w_qkv = np.asarray(w_qkv, np.float32)
    b_qkv = np.asarray(b_qkv, np.float32)
    w_proj = np.asarray(w_proj, np.float32)
    maskT_bf = np.ascontiguousarray((np.asarray(attn_mask).T != 0)).astype(
        ml_dtypes.bfloat16)
    maps = []
    for c in range(NCORES):
        b = c // 2
        col0 = (c % 2) * (HPC * HD)  # 0 or 512
        maps.append({
            "xT": np.ascontiguousarray(x[b].T),
            "w_qk": np.ascontiguousarray(
                np.concatenate([w_qkv[:, col0:col0 + 512],
                                w_qkv[:, C + col0:C + col0 + 512]], axis=1)),
            "w_v": np.ascontiguousarray(w_qkv[:, 2 * C + col0:2 * C + col0 + 512]),
            "b_qk": np.ascontiguousarray(
                np.concatenate([b_qkv[col0:col0 + 512],
                                b_qkv[C + col0:C + col0 + 512]])),
            "b_v": np.ascontiguousarray(b_qkv[2 * C + col0:2 * C + col0 + 512]),
            "maskT": maskT_bf,
            "w_proj": np.ascontiguousarray(
                w_proj[col0:col0 + 512, :]).astype(ml_dtypes.bfloat16),
        })
    return maps


def run_cores(in_maps, **kw):
    return run_bass_kernel_spmd(_get_nc(), in_maps, core_ids=list(range(NCORES)), **kw)


def _gather(results, attn_mask, b_proj):
    b_proj = np.asarray(b_proj, np.float32)
    mask_f = (np.asarray(attn_mask) != 0).astype(np.float32)
    attn = np.empty((B, HEADS, N, N), np.float32)
    out = np.empty((B, N, C), np.float32)
    for c in range(NCORES):
        b = c // 2
        h0 = (c % 2) * HPC
        blk = attn[b, h0:h0 + HPC]
        blk[:] = results[c]["attn_out"]          # E = exp(S*scale), bf16->f32
        # d_out layout [h, p, qc] -> D[h, q] with q = qc*128 + p
        D = results[c]["d_out"].transpose(0, 2, 1).reshape(HPC, N)
        blk *= (1.0 / D)[:, :, None]
        blk *= mask_f[None, :, :]
    for b in range(B):
        out[b] = results[2 * b]["y_out"] + results[2 * b + 1]["y_out"] + b_proj[None, :]
    return out, attn


def kernel(x, attn_mask, w_qkv, b_qkv, w_proj, b_proj):
    maps = _in_maps(x, attn_mask, w_qkv, b_qkv, w_proj, b_proj)
    res = run_cores(maps)
    return _gather(res.results, attn_mask, b_proj)
